# revision 22
# baseline (speedup 1.0000x reference)
"""Block-sparse flash attention (Phi-3-small pattern) on 8 Trainium2 cores.

Problem: S=2048 tokens, 32 query heads, 8 KV heads (GQA x4), D=128,
sparse_block_size=64, local_blocks=16, vert_stride=8, per-head vertical
offset (homo_head=False).

Sharding: tensor-parallel over heads. Core r owns contiguous heads
[4r, 4r+4), which all share GQA KV head r.

Per-head block mask (head h, c = (7-h) % 8):
  block (qb, kb) active iff qb >= kb and (qb-kb < 16 or kb % 8 == c)
Decomposition (exact vs reference):
  - LOCAL pass, k-tile kt (128 k rows): q in [128kt, 128kt+1088)
      * causal triangle on the diagonal 128 cols, applied as a RAMP
        matmul into PSUM: scores += -150*(k-q)+ before exp, so masked
        entries exp to ~0 (<= e^-8, negligible vs rowsums ~1e2).
      * zero k-rows [0:64) of the last 64 q cols (qb-kb == 16 corner)
  - TAIL pass: vertical kbs {c, c+8} gathered on host into one 128-row
    k-tile; q in [1024, 2048) with a per-head 0/1 mask (tm input).

v13 design (v12 trace showed a latency bound: the 2-slot score ring
serializes QK->sem->exp->sem at ~1.3us/unit):
  - Each (pair, half) runs in TWO PHASES, one per 512-col output
    window, so the PV accumulator needs only [128, 2, 512] (2 PSUM
    banks) and the score ring gets THREE [128, 2, 512] slots.
  - Heads in PAIRS: one wide exp per unit; ~40-50% of non-diagonal
    exp units run on DVE as Schraudolph fast-exp (tensor_scalar ->
    int16 bitcast fp16, +-3%), alternating with ACT.
  - Diagonal masking via ramp matmul on PE (start=True zeroes the
    2KB PSUM row, QK accumulates start=False).
  - Large steps' eT unit tiles are DMA'd to DRAM (inline, right after
    masks) and their rowsum contribution is summed on the HOST,
    removing most DVE accumulate traffic.
  - First-coverage step of each half writes exp output directly into
    acc2; PV reads acc2 for those units.
  - Per-phase epilogue: ACT copy [128,2,512] -> SBUF -> 2 DMAs.
  - Per-half rowsum: 2 ones-matmuls per head right after the half's
    last accumulate; host adds dumped colsums and divides.
  - GpSimd issues input DMAs + half the dump DMAs; corner memsets and
    tail masks on DVE.

All per-head pattern differences are input DATA (kvT/vv/tm), so the
single SPMD program is identical on all 8 cores.
"""

import sys
from contextlib import ExitStack

import numpy as np

for _p in ("/opt/trn_rl_repo", "/root/.axon_site/_ro/trn_rl_repo"):
    if _p not in sys.path:
        sys.path.append(_p)

import concourse.bass as bass
import concourse.bacc as bacc
import concourse.mybir as mybir
import concourse.tile as tile
from concourse.bass_utils import run_bass_kernel_spmd

S = 2048
D = 128
H = 32
HKV = 8
NCORES = 8
NH = H // NCORES          # heads per core = 4
NP = NH // 2              # head pairs per core = 2
SCALE = 0.08838834764831845
NKT = S // 128            # 16 k-tiles of 128 rows
SPAN = 1088               # local window cols per k-tile (17 blocks of 64)
HALF = 1024
WIN = 512                 # PSUM bank window

F16 = mybir.dt.float16
F32 = mybir.dt.float32
I16 = mybir.dt.int16
NPF16 = np.float16

RAMP_C = 150.0            # ramp slope: masked r>=1 -> exp <= e^-8

# Schraudolph fast-exp constants (fp16 bit trick, trunc-to-int16):
#   exp(SCALE*x) ~= bitcast_f16(int16(x*SCH_A + SCH_B)), +-3% rel err
SCH_A = (1024.0 / np.log(2.0)) * SCALE
SCH_B = 15360.0 - 44.5

DUMP_MIN = 640            # non-fresh steps with span >= this go to host


def half_steps(half):
    """Step list for one q-half: (kind, kt, a, b) with [a,b) the q span."""
    half_lo = HALF * half
    half_hi = half_lo + HALF
    steps = []
    if half == 1:
        steps.append(("tail", -1, HALF, S))
    for kt in range(NKT):
        a = max(128 * kt, half_lo)
        b = min(128 * kt + SPAN, half_hi)
        if a < b:
            steps.append(("loc", kt, a, b))
    return steps


def phase_units(half, w):
    """Units (si, kind, kt, a, b, qlo, qhi) of window phase w, step order."""
    wlo = HALF * half + WIN * w
    whi = wlo + WIN
    out = []
    for si, (kind, kt, a, b) in enumerate(half_steps(half)):
        qlo, qhi = max(a, wlo), min(b, whi)
        if qlo < qhi:
            out.append((si, kind, kt, a, b, qlo, qhi))
    return out


def dump_schedule():
    """(pair, a(unit qlo), b(unit qhi)) of dumped units, program order."""
    out = []
    for p in range(NP):
        for half in (0, 1):
            for w in (0, 1):
                for (si, kind, kt, a, b, qlo, qhi) in phase_units(half, w):
                    if si > 0 and b - a >= DUMP_MIN:
                        out.append((p, qlo, qhi))
    return out


DUMPS = dump_schedule()
NDUMP = len(DUMPS)


def build_program(lag=8, eTd=12, dve_mod=2, dve_cnt=1):
    nc = bacc.Bacc("TRN2", target_bir_lowering=False, debug=False)
    qT = nc.dram_tensor("qT", [NH, 128, S], F16, kind="ExternalInput").ap()
    kT = nc.dram_tensor("kT", [128, S], F16, kind="ExternalInput").ap()
    vR = nc.dram_tensor("vR", [128, S], F16, kind="ExternalInput").ap()
    kvT = nc.dram_tensor("kvT", [NH, 128, 128], F16, kind="ExternalInput").ap()
    vv = nc.dram_tensor("vv", [NH, 128, 128], F16, kind="ExternalInput").ap()
    tm2 = nc.dram_tensor("tm2", [NP, 128, 2, HALF], F16,
                         kind="ExternalInput").ap()
    rampL = nc.dram_tensor("rampL", [128, 128], F16, kind="ExternalInput").ap()
    rampR = nc.dram_tensor("rampR", [128, 128], F16, kind="ExternalInput").ap()
    outT = nc.dram_tensor("outT", [NH, 128, S], F16, kind="ExternalOutput").ap()
    # per (head, half): rowsum rows {0,32} over that half's 2x512 cols
    rsD = nc.dram_tensor("rs", [NH, 2, 64, WIN], F16,
                         kind="ExternalOutput").ap()
    eTD = nc.dram_tensor("eTd", [max(NDUMP, 1), 128, 2, WIN], F16,
                         kind="ExternalOutput").ap()

    Exp = mybir.ActivationFunctionType.Exp
    Copy = mybir.ActivationFunctionType.Copy
    MUL = mybir.AluOpType.mult
    ADD = mybir.AluOpType.add

    with tile.TileContext(nc) as tc, ExitStack() as ctx:
        const = ctx.enter_context(tc.tile_pool(name="const", bufs=1))

        # ---- persistent SBUF tiles ----
        kT_sb = const.tile([128, S], F16, tag="kT")
        v_sb = const.tile([128, S], F16, tag="v")
        rampL_sb = const.tile([128, 128], F16, tag="rampL")
        rampR2_sb = const.tile([128, 2, 128], F16, tag="rampR")
        qT_sb = [const.tile([128, S], F16, tag=f"qT{h}", name=f"qT{h}")
                 for h in range(NH)]
        kvT_sb = [const.tile([128, 128], F16, tag=f"kvT{h}", name=f"kvT{h}")
                  for h in range(NH)]
        vv_sb = [const.tile([128, 128], F16, tag=f"vv{h}", name=f"vv{h}")
                 for h in range(NH)]
        tm_sb = [const.tile([128, 2, HALF], F16, tag=f"tm{p}", name=f"tm{p}")
                 for p in range(NP)]
        acc2 = [const.tile([128, 2, S], F16, tag=f"acc{p}", name=f"acc{p}")
                for p in range(NP)]
        ones_sb = const.tile([128, 32], F16, tag="ones")
        nc.vector.memset(ones_sb[:], 1.0)

        # ---- input DMAs on the GpSimd queue (cheap issue), JIT order ----
        # startup-critical DMAs all on the GpSimd queue: the Sync queue is
        # occupied by the SPMD start barrier for the first ~3us
        nc.gpsimd.dma_start(kT_sb[:, 0:WIN], kT[:, 0:WIN])
        nc.gpsimd.dma_start(qT_sb[0][:, 0:WIN], qT[0][:, 0:WIN])
        nc.gpsimd.dma_start(qT_sb[1][:, 0:WIN], qT[1][:, 0:WIN])
        nc.gpsimd.dma_start(v_sb[:, 0:WIN], vR[:, 0:WIN])
        nc.gpsimd.dma_start(rampL_sb[:], rampL[:])
        nc.gpsimd.dma_start(rampR2_sb[:, 0, :], rampR[:])
        nc.gpsimd.dma_start(rampR2_sb[:, 1, :], rampR[:])
        nc.gpsimd.dma_start(kT_sb[:, WIN:HALF], kT[:, WIN:HALF])
        nc.gpsimd.dma_start(qT_sb[0][:, WIN:HALF], qT[0][:, WIN:HALF])
        nc.gpsimd.dma_start(qT_sb[1][:, WIN:HALF], qT[1][:, WIN:HALF])
        nc.gpsimd.dma_start(v_sb[:, WIN:HALF], vR[:, WIN:HALF])
        nc.gpsimd.dma_start(kT_sb[:, HALF:S], kT[:, HALF:S])
        nc.gpsimd.dma_start(qT_sb[0][:, HALF:S], qT[0][:, HALF:S])
        nc.gpsimd.dma_start(qT_sb[1][:, HALF:S], qT[1][:, HALF:S])
        nc.gpsimd.dma_start(v_sb[:, HALF:S], vR[:, HALF:S])
        nc.gpsimd.dma_start(kvT_sb[0][:], kvT[0])
        nc.gpsimd.dma_start(kvT_sb[1][:], kvT[1])
        nc.gpsimd.dma_start(vv_sb[0][:], vv[0])
        nc.gpsimd.dma_start(vv_sb[1][:], vv[1])
        nc.gpsimd.dma_start(tm_sb[0][:], tm2[0])

        eTp = ctx.enter_context(tc.tile_pool(name="eT", bufs=eTd))
        osbp = ctx.enter_context(tc.tile_pool(name="osb", bufs=3))
        rscp = ctx.enter_context(tc.tile_pool(name="rsc", bufs=2))
        scp = ctx.enter_context(tc.tile_pool(name="scores", bufs=3,
                                             space="PSUM"))
        otp = ctx.enter_context(tc.tile_pool(name="outw", bufs=1,
                                             space="PSUM"))

        # preload the ACT Exp table during input DMAs
        warm = const.tile([128, 1], F32, tag="warm")
        nc.vector.memset(warm[:], 0.0)
        warm2 = const.tile([128, 1], F16, tag="warm2")
        nc.scalar.activation(warm2[:], warm[:], Exp)

        pending = []

        def flush_one(force=False):
            if pending and (force or len(pending) > lag):
                pending.pop(0)()

        ucount = [0]
        dump_idx = [0]
        dump_q = [0]

        for p in range(NP):
            h0, h1 = 2 * p, 2 * p + 1
            if p + 1 < NP:
                for hn in (2 * p + 2, 2 * p + 3):
                    nc.gpsimd.dma_start(qT_sb[hn][:], qT[hn])
                    nc.gpsimd.dma_start(kvT_sb[hn][:], kvT[hn])
                    nc.gpsimd.dma_start(vv_sb[hn][:], vv[hn])
                nc.gpsimd.dma_start(tm_sb[p + 1][:], tm2[p + 1])
            for half in (0, 1):
                half_lo = HALF * half
                for w in (0, 1):
                    wlo = half_lo + WIN * w
                    units = phase_units(half, w)
                    nu = len(units)
                    ow = otp.tile([128, 2, WIN], F32, tag="ow", name="ow")
                    seen = [0]

                    for (si, kind, kt, a, b, qlo, qhi) in units:
                        fresh = si == 0
                        dump = (not fresh) and (b - a >= DUMP_MIN)
                        n = qhi - qlo
                        flush_one()
                        if kind == "loc":
                            lhs_qk = [kT_sb[:, 128 * kt:128 * kt + 128]] * 2
                            lhs_pv = [v_sb[:, 128 * kt:128 * kt + 128]] * 2
                            diag = 128 * kt if kt // 8 == half else -1
                            cor = (b if kt <= 7 and b == 128 * kt + SPAN
                                   else -1)
                            is_tail = False
                        else:
                            lhs_qk = [kvT_sb[h0][:], kvT_sb[h1][:]]
                            lhs_pv = [vv_sb[h0][:], vv_sb[h1][:]]
                            diag = cor = -1
                            is_tail = True
                        has_diag = diag >= 0 and qlo <= diag < qhi

                        eT = None
                        if not fresh:
                            eT = eTp.tile([128, 2, WIN], F16, tag="eT")

                        sc = scp.tile([128, 2, WIN], F32, tag="sc")
                        for i in (0, 1):
                            if has_diag:
                                nc.tensor.matmul(
                                    sc[:, i, 0:128], rampL_sb[:],
                                    rampR2_sb[:, 0, :], start=True,
                                    stop=False, skip_group_check=True)
                            nc.tensor.matmul(
                                sc[:, i, 0:n], lhs_qk[i],
                                qT_sb[(h0, h1)[i]][:, qlo:qhi],
                                start=not has_diag, stop=True,
                                skip_group_check=True)
                        if fresh:
                            dst = acc2[p][:, :, qlo:qhi]
                        else:
                            dst = eT[:, :, 0:n]
                        use_dve = False
                        if not has_diag:
                            uc = ucount[0]
                            ucount[0] += 1
                            use_dve = uc % dve_mod < dve_cnt
                        if use_dve:
                            nc.vector.tensor_scalar(
                                dst.bitcast(I16), sc[:, :, 0:n],
                                SCH_A, SCH_B, op0=MUL, op1=ADD)
                        else:
                            nc.scalar.activation(dst, sc[:, :, 0:n], Exp,
                                                 scale=SCALE)
                        if cor >= 0 and qhi == cor:
                            nc.vector.memset(dst[0:64, :, n - 64:n], 0.0)
                        if is_tail:
                            t0 = qlo - HALF
                            nc.vector.tensor_mul(
                                dst, dst, tm_sb[p][:, :, t0:t0 + n])
                        if dump:
                            di = dump_idx[0]
                            dump_idx[0] += 1
                            dq = nc.sync if dump_q[0] == 0 else nc.gpsimd
                            dump_q[0] ^= 1
                            dq.dma_start(eTD[di][:, :, 0:n], eT[:, :, 0:n])

                        def stage_b(p=p, fresh=fresh, dump=dump,
                                    lhs_pv=lhs_pv, eT=eT,
                                    ow=ow, qlo=qlo, qhi=qhi, wlo=wlo,
                                    seen=seen, nu=nu):
                            # acc add stays here: it must execute after the
                            # fresh unit's PV read of the same acc2 region
                            if not fresh and not dump:
                                nc.vector.tensor_add(
                                    acc2[p][:, :, qlo:qhi],
                                    acc2[p][:, :, qlo:qhi],
                                    eT[:, :, 0:qhi - qlo])
                            seen[0] += 1
                            st = seen[0] == 1
                            sp = seen[0] == nu
                            for i in (0, 1):
                                if fresh:
                                    rhs = acc2[p][:, i, qlo:qhi]
                                else:
                                    rhs = eT[:, i, 0:qhi - qlo]
                                nc.tensor.matmul(
                                    ow[:, i, qlo - wlo:qhi - wlo],
                                    lhs_pv[i], rhs,
                                    start=st, stop=sp,
                                    skip_group_check=True)

                        pending.append(stage_b)

                    def phase_epilogue(p=p, wlo=wlo, ow=ow):
                        osb = osbp.tile([128, 2, WIN], F16, tag="os",
                                        name="osb")
                        nc.scalar.activation(osb[:], ow[:], Copy)
                        for i in (0, 1):
                            nc.sync.dma_start(
                                outT[2 * p + i][:, wlo:wlo + WIN],
                                osb[:, i, :])

                    pending.append(phase_epilogue)

                def half_rs(p=p, half=half, half_lo=half_lo):
                    rs4 = scp.tile([128, 2, WIN], F32, tag="sc", name="rs4")
                    for i in (0, 1):
                        for j in (0, 1):
                            q0 = half_lo + WIN * j
                            nc.tensor.matmul(
                                rs4[32 * j:32 * j + 32, i, 0:WIN],
                                ones_sb[:], acc2[p][:, i, q0:q0 + WIN],
                                start=True, stop=True,
                                tile_position=(0, 32 * j) if j else None)
                    rsc = rscp.tile([64, 2, WIN], F16, tag="rsc", name="rsc")
                    nc.scalar.activation(rsc[:], rs4[0:64, :, :], Copy)
                    for i in (0, 1):
                        nc.sync.dma_start(rsD[2 * p + i, half], rsc[:, i, :])

                pending.append(half_rs)

        while pending:
            flush_one(force=True)
    nc.compile()
    return nc


def make_core_inputs(query, key, value, core):
    """Host-side prep of one core's input map (fp16, pre-transposed/gathered)."""
    q3 = query.reshape(S, H, D)
    k3 = key.reshape(S, HKV, D)
    v3 = value.reshape(S, HKV, D)
    r = core
    K = k3[:, r, :]                     # [S, 128]
    V = v3[:, r, :]
    KT = np.ascontiguousarray(K.T)      # [128, S]
    vRe = np.ascontiguousarray(
        V.reshape(NKT, 128, D).transpose(1, 0, 2).reshape(128, S))

    qT = np.empty((NH, 128, S), NPF16)
    kvT = np.empty((NH, 128, 128), NPF16)
    vv = np.empty((NH, 128, 128), NPF16)
    tm2 = np.zeros((NP, 128, 2, HALF), NPF16)
    for hl in range(NH):
        hg = NH * r + hl
        c = (7 - hg) % 8
        qT[hl] = q3[:, hg, :].T.astype(NPF16)
        kvT[hl, :, 0:64] = KT[:, 64 * c:64 * c + 64].astype(NPF16)
        kvT[hl, :, 64:128] = KT[:, 64 * (c + 8):64 * (c + 8) + 64].astype(NPF16)
        vv[hl, 0:64, :] = V[64 * c:64 * c + 64, :].astype(NPF16)
        vv[hl, 64:128, :] = V[64 * (c + 8):64 * (c + 8) + 64, :].astype(NPF16)
        qq = np.arange(HALF)
        tm2[hl // 2, 0:64, hl % 2, :] = (qq >= 64 * c).astype(NPF16)[None, :]
        tm2[hl // 2, 64:128, hl % 2, :] = (qq >= 512 + 64 * c).astype(NPF16)[None, :]

    dd = np.arange(128)
    # rampL[d, k] = -RAMP_C * [d < k];  rampR[d, q] = [d >= q]
    # => (rampL.T @ rampR)[k, q] = -RAMP_C * max(0, k - q)
    rampL = (-RAMP_C * (dd[:, None] < dd[None, :])).astype(NPF16)
    rampR = (dd[:, None] >= dd[None, :]).astype(NPF16)

    return {
        "qT": qT,
        "kT": KT.astype(NPF16),
        "vR": vRe.astype(NPF16),
        "kvT": kvT,
        "vv": vv,
        "tm2": tm2,
        "rampL": rampL,
        "rampR": rampR,
    }


_PROGRAM = None


def _get_program():
    global _PROGRAM
    if _PROGRAM is None:
        _PROGRAM = build_program()
    return _PROGRAM


def run(query, key, value, trace=False):
    """Returns (output [S, H*D] f32, BassKernelResults)."""
    nc = _get_program()
    in_maps = [make_core_inputs(query, key, value, r) for r in range(NCORES)]
    br = run_bass_kernel_spmd(nc, in_maps, list(range(NCORES)), trace=trace)
    # host epilogue: outT [NH, 128, S] -> out[q, d] / rs[q]
    outs = []
    for r in range(NCORES):
        oT = br.results[r]["outT"].astype(np.float32)   # [NH, 128, S]
        rs = br.results[r]["rs"].astype(np.float32)     # [NH, 2, 64, WIN]
        rsq = rs[:, :, [0, 32], :].reshape(NH, S)       # [NH, S]
        # add host-side rowsum contributions from dumped eT unit tiles
        eTd = br.results[r]["eTd"].astype(np.float32)   # [NDUMP,128,2,WIN]
        for di, (p, qlo, qhi) in enumerate(DUMPS):
            colsum = eTd[di, :, :, 0:qhi - qlo].sum(axis=0)  # [2, n]
            rsq[2 * p, qlo:qhi] += colsum[0]
            rsq[2 * p + 1, qlo:qhi] += colsum[1]
        o = oT.transpose(2, 0, 1) / rsq.T[:, :, None]   # [S, NH, 128]
        outs.append(o.reshape(S, NH * D))
    outp = np.hstack(outs).astype(np.float32)
    return outp, br


def kernel(query, key, value):
    outp, _ = run(np.asarray(query), np.asarray(key), np.asarray(value))
    return outp
